# revision 2
# baseline (speedup 1.0000x reference)
"""Trainium2 Bass kernel v3 for nn_Decoder (recursive tree GRU decoder).

Self-contained: builds + compiles + runs a Bass/Tile kernel SPMD on 8
NeuronCores, pure data-parallel over the batch dim.

Design:
  - bf16 activations/weights everywhere (DVE 2x/4x modes on SBUF ops, FWL on
    PE, half DMA traffic); PSUM stays fp32.
  - 4 independent streams per core (1024 batch each = 2 tiles of 512), each
    running the tree recursion; streams interleaved at emission so all five
    engines pipeline across streams.
  - GRU elementwise per stream (FD=1024 ops): gates [A|A],[B|B],[C|C],[D|D]
    in 2-bank PSUM tiles; ACT: 2x tanh(gate/2) + 1x tanh(n); DVE:
    m=(trzA+1)*D [1x], na=m+C [1x], z=(trzB+1)/2 [4x ts], pz=z*s [2x],
    h'=pz+n [2x]; s=h-n on GPSIMD (otherwise idle).
  - Output: exp(pred) bf16 DMA'd feature-major (exp is also the softmax
    numerator, so the logits copy is free); host takes log + transposes.
  - z input loaded bf16 via DMA xbar transpose.

Math identical to reference (tanh-only gates: sigmoid(x)=(tanh(x/2)+1)/2,
wh2 pre-scaled by 0.5 on host).
"""

import numpy as np
import ml_dtypes

import concourse.bass as bass
import concourse.mybir as mybir
from concourse import tile
from concourse.bass_utils import run_bass_kernel_spmd

F32 = mybir.dt.float32
BF16 = mybir.dt.bfloat16
AF = mybir.ActivationFunctionType
ALU = mybir.AluOpType

B, I, H, O = 32768, 128, 128, 32
N_CORES = 8
BT = 512            # batch tile (one PSUM bank of fp32)
NT = 2              # tiles per stream (NT*O = 64 pred partitions)
B_CORE = B // N_CORES            # 4096
NS = B_CORE // (BT * NT)         # 4 streams per core
PO = NT * O                      # pred partitions (64)

# weight layout columns
W_Z2H = 0
W_S = 128
W_H2O = W_S + 64
GRU_STRIDE = 3 * NT * 128 + 3 * 128   # wi blocks + wh blocks = 1152
W_GRU0 = W_H2O + NT * 64
W_U0 = W_GRU0 + 2 * GRU_STRIDE
W_ID = W_U0 + 256
W_COLS = W_ID + 128

POOL_S = True       # run the GRU `s = h - n` op on GPSIMD instead of DVE
PE_ACCUM_NA = False  # PE identity-accumulate m into XC (kills `na` DVE op)
TP_BUFS = 4
HP_BUFS = 2

_PE_OPS = ("InstMatmult", "InstLdweights", "InstMatmultMx")


def _split_multi_waits(nc):
    """This container's walrus accepts at most 1 embedded sem wait on most
    instructions (0 on self-loading matmuls) and <=2 on a standalone
    EventSemaphore.  Tile emits multi-waits; split them."""
    for f in nc.m.functions:
        for bb in f.blocks:
            insts = bb.instructions
            new = []
            changed = False
            for ins in insts:
                si = ins.sync_info
                ow = list(si.on_wait) if si is not None and si.on_wait else []
                movable = [w for w in ow if w.wait_reg is None]
                fixed = [w for w in ow if w.wait_reg is not None]
                opc = type(ins).__name__
                limit = 0 if opc in _PE_OPS else 1
                limit = max(0, limit - len(fixed))
                if len(movable) > limit:
                    keep = movable[:limit]
                    move = movable[limit:]
                    for i in range(0, len(move), 2):
                        ev = mybir.InstEventSemaphore(
                            name=f"{ins.name}-wsp{i}",
                            ins=[],
                            outs=[],
                            sync_info=mybir.SyncInfo(
                                on_wait=move[i : i + 2], on_update=[]
                            ),
                        )
                        ev.engine = ins.engine
                        new.append(ev)
                    upd = list(si.on_update) if si.on_update else []
                    ins.sync_info = mybir.SyncInfo(on_wait=fixed + keep, on_update=upd)
                    changed = True
                new.append(ins)
            if changed:
                bb.instructions = new


def _n_nodes(depth, arity):
    n, level = 0, 1
    for _ in range(depth + 1):
        n += level
        level *= arity
    return n


def build(depth, arity, loop_n=1):
    """Build the per-core Bass module.  Returns (nc, n_nodes)."""
    nn_ = _n_nodes(depth, arity)
    nc = bass.Bass(trn_type="TRN2")

    z_d = nc.dram_tensor("z", [B_CORE, I], BF16, kind="ExternalInput")
    w_d = nc.dram_tensor("w_all", [128, W_COLS], BF16, kind="ExternalInput")
    # per node per stream: exp(pred) [part = 2t x 32o, col = batch-in-tile]
    out_d = nc.dram_tensor("out", [nn_, NS, PO, BT], BF16,
                           kind="ExternalOutput")

    with tile.TileContext(nc) as tc:
        with (
            tc.tile_pool(name="wp", bufs=1) as wp,
            tc.tile_pool(name="hp", bufs=HP_BUFS) as hp,
            tc.tile_pool(name="prp", bufs=1) as prp,
            tc.tile_pool(name="tp", bufs=TP_BUFS) as tp,
            tc.tile_pool(name="pp", bufs=3, space="PSUM") as pp,
            tc.tile_pool(name="p1", bufs=2, space="PSUM") as p1,
        ):
            w = wp.tile([128, W_COLS], BF16, tag="w_all")
            nc.sync.dma_start(w[:], w_d[:])

            w_z2h = w[:, W_Z2H : W_Z2H + 128]
            w_S2 = w[0:PO, W_S : W_S + PO]
            def w_h2o(t):
                return w[:, W_H2O + t * PO : W_H2O + (t + 1) * PO]
            def w_gi(g, k, t):
                base = W_GRU0 + g * GRU_STRIDE + (k * NT + t) * 128
                return w[0:PO, base : base + 128]
            def w_gh(g, k):
                base = W_GRU0 + g * GRU_STRIDE + 3 * NT * 128 + k * 128
                return w[:, base : base + 128]
            w_uf = w[:, W_U0 : W_U0 + 128]
            w_ua = w[:, W_U0 + 128 : W_U0 + 256]
            w_id = w[:, W_ID : W_ID + 128]

            from contextlib import ExitStack
            _ls = ExitStack()
            _ls.enter_context(nc.allow_low_precision(reason="bf16 decoder"))
            if loop_n > 1:
                _ls.enter_context(tc.For_i(0, loop_n, 1))

            def stream_gen(sid):
                node_idx = [0]

                # ---- hidden0 = z @ z2h_w  (feature-major) ----
                h_ps = pp.tile([128, 2 * BT], F32, tag="g2")
                for i in range(2):
                    base = sid * (NT * BT) + i * BT
                    zT = tp.tile([128, BT], BF16, tag="zT")
                    nc.sync.dma_start_transpose(zT[:], z_d[base : base + BT, :])
                    nc.tensor.matmul(
                        h_ps[:, i * BT : (i + 1) * BT], w_z2h, zT[:],
                        start=True, stop=True,
                    )
                h0 = hp.tile([128, 2 * BT], BF16, tag=f"h_s{sid}_d{depth}")
                nc.scalar.copy(h0[:], h_ps[:])

                def pred_softmax(h, d, need_probs):
                    n = node_idx[0]
                    node_idx[0] += 1
                    ps = p1.tile([PO, BT], F32, tag="p1")
                    pred_ps = ps[:]
                    for t in range(NT):
                        nc.tensor.matmul(
                            pred_ps, w_h2o(t),
                            h[:, t * BT : (t + 1) * BT],
                            start=(t == 0), stop=(t == NT - 1),
                        )
                    # exp(pred) is both the output (host takes log) and the
                    # softmax numerator.
                    exp_sb = tp.tile([PO, BT], BF16, tag="exp_sb")
                    nc.scalar.activation(exp_sb[:], pred_ps, AF.Exp,
                                         bias=0.0, scale=1.0)
                    probs = None
                    if need_probs:
                        sums_t = p1.tile([PO, BT], F32, tag="p1")
                        sums_ps = sums_t[:]
                        nc.tensor.matmul(sums_ps, w_S2, exp_sb[:],
                                         start=True, stop=True)
                        rbc = tp.tile([PO, BT], BF16, tag="rbc")
                        nc.vector.reciprocal(rbc[:], sums_ps)
                        probs = prp.tile([PO, BT], BF16, tag=f"probs_s{sid}_d{d}")
                        nc.vector.tensor_tensor(out=probs[:], in0=exp_sb[:],
                                                in1=rbc[:], op=ALU.mult)
                    nc.sync.dma_start(out_d[n, sid], exp_sb[:])
                    return probs

                def gru(g, probs, h, d):
                    # gates (fp32 PSUM, [128, 1024] = 2 tiles of 512):
                    #   XA = gi0 + gh0 ; XB = gi1 + gh1 ; XC = gi2 ; XD = 0.5*gh2
                    # r-path: n = tanh(gi2 + r*gh2) = tanh((trzA+1)*XD + XC)
                    # z-path: z = (trzB+1)/2 ; h' = n + z*(h - n)
                    # fill order A, D, C, B: consumers chain trzA -> m(XD) ->
                    # na(XC), and trzB is needed only late (z-path), so this
                    # minimizes PSUM slot residency.
                    XA = pp.tile([128, 2 * BT], F32, tag="g2")
                    for i in range(2):
                        hsl = h[:, i * BT : (i + 1) * BT]
                        c = slice(i * BT, (i + 1) * BT)
                        nc.tensor.matmul(XA[:, c], w_gi(g, 0, i), probs[:], start=True, stop=False)
                        nc.tensor.matmul(XA[:, c], w_gh(g, 0), hsl, start=False, stop=True)
                    trzA = tp.tile([128, 2 * BT], BF16, tag="trzA")
                    nc.scalar.activation(trzA[:], XA[:], AF.Tanh, bias=0.0, scale=0.5)
                    XD = pp.tile([128, 2 * BT], F32, tag="g2")
                    for i in range(2):
                        hsl = h[:, i * BT : (i + 1) * BT]
                        c = slice(i * BT, (i + 1) * BT)
                        nc.tensor.matmul(XD[:, c], w_gh(g, 2), hsl, start=True, stop=True)
                    XC = pp.tile([128, 2 * BT], F32, tag="g2")
                    for i in range(2):
                        c = slice(i * BT, (i + 1) * BT)
                        nc.tensor.matmul(XC[:, c], w_gi(g, 2, i), probs[:],
                                         start=True, stop=not PE_ACCUM_NA)
                    XB = pp.tile([128, 2 * BT], F32, tag="g2")
                    for i in range(2):
                        hsl = h[:, i * BT : (i + 1) * BT]
                        c = slice(i * BT, (i + 1) * BT)
                        nc.tensor.matmul(XB[:, c], w_gi(g, 1, i), probs[:], start=True, stop=False)
                        nc.tensor.matmul(XB[:, c], w_gh(g, 1), hsl, start=False, stop=True)
                    trzB = tp.tile([128, 2 * BT], BF16, tag="trzB")
                    nc.scalar.activation(trzB[:], XB[:], AF.Tanh, bias=0.0, scale=0.5)
                    m = tp.tile([128, 2 * BT], BF16, tag="m")
                    nc.vector.scalar_tensor_tensor(
                        out=m[:], in0=trzA[:], scalar=1.0, in1=XD[:],
                        op0=ALU.add, op1=ALU.mult,
                    )
                    if PE_ACCUM_NA:
                        # accumulate m into XC on the PE (identity stationary):
                        # XC <- gi2 + m, so nn = tanh(XC) straight from PSUM.
                        # (costs PE head-of-line blocking; off by default)
                        for i in range(2):
                            c = slice(i * BT, (i + 1) * BT)
                            nc.tensor.matmul(XC[:, c], w_id, m[:, c], start=False, stop=True)
                        na_src = XC[:]
                    else:
                        na = tp.tile([128, 2 * BT], BF16, tag="na")
                        nc.vector.tensor_tensor(out=na[:], in0=m[:], in1=XC[:], op=ALU.add)
                        na_src = na[:]
                    nn_t = tp.tile([128, 2 * BT], BF16, tag="nn_t")
                    nc.scalar.activation(nn_t[:], na_src, AF.Tanh, bias=0.0, scale=1.0)
                    zg = tp.tile([128, 2 * BT], BF16, tag="zg")
                    nc.vector.tensor_scalar(zg[:], trzB[:], 0.5, 0.5,
                                            op0=ALU.mult, op1=ALU.add)
                    s = tp.tile([128, 2 * BT], BF16, tag="s")
                    s_eng = nc.gpsimd if POOL_S else nc.vector
                    s_eng.tensor_tensor(out=s[:], in0=h[:], in1=nn_t[:], op=ALU.subtract)
                    pz = tp.tile([128, 2 * BT], BF16, tag="pz")
                    nc.vector.tensor_tensor(out=pz[:], in0=zg[:], in1=s[:], op=ALU.mult)
                    hn = hp.tile([128, 2 * BT], BF16, tag=f"h_s{sid}_d{d}")
                    nc.vector.tensor_tensor(out=hn[:], in0=pz[:], in1=nn_t[:], op=ALU.add)
                    return hn

                def u_stage(hf, ha, d):
                    U = pp.tile([128, 2 * BT], F32, tag="g2")
                    for i in range(2):
                        c = slice(i * BT, (i + 1) * BT)
                        nc.tensor.matmul(U[:, c], w_uf, hf[:, c], start=True, stop=False)
                        nc.tensor.matmul(U[:, c], w_ua, ha[:, c], start=False, stop=True)
                    ht = hp.tile([128, 2 * BT], BF16, tag=f"h_s{sid}_d{d}")
                    nc.scalar.activation(ht[:], U[:], AF.Tanh, bias=0.0, scale=1.0)
                    return ht

                def rec(h, d, need_probs):
                    probs = pred_softmax(h, d, need_probs or d > 0)
                    yield
                    if d == 0:
                        return probs
                    h1 = gru(0, probs, h, d - 1)
                    yield
                    probs_f = yield from rec(h1, d - 1, arity > 1)
                    hf = h1
                    for sI in range(arity - 1):
                        hf = gru(1, probs_f, hf, d - 1)
                        yield
                        h2 = u_stage(hf, h, d - 1)
                        yield
                        probs_f = yield from rec(h2, d - 1, sI < arity - 2)
                    return probs

                yield from rec(h0, depth, False)

            gens = [stream_gen(s) for s in range(NS)]
            live = list(gens)
            while live:
                for g in list(live):
                    try:
                        next(g)
                    except StopIteration:
                        live.remove(g)

            _ls.close()

    _split_multi_waits(nc)
    return nc, nn_


def _prep_weights(inputs):
    """Host-side weight packing into one [128, W_COLS] bf16 tensor."""
    f = lambda x: np.asarray(x, dtype=np.float32)
    z2h_w = f(inputs["z2h_w"])            # [I, H]
    h2o_w = f(inputs["h2o_w"])            # [H, O]
    w_all = np.zeros((128, W_COLS), np.float32)
    w_all[:, W_Z2H : W_Z2H + 128] = z2h_w
    for t in range(NT):
        w_all[t * O : (t + 1) * O, W_S + t * O : W_S + (t + 1) * O] = 1.0
    for t in range(NT):
        w_all[:, W_H2O + t * PO + t * O : W_H2O + t * PO + (t + 1) * O] = h2o_w
    for g, name in enumerate(("anc", "frat")):
        wi = f(inputs[f"{name}_wi"])      # [3, O, H]
        wh = f(inputs[f"{name}_wh"])      # [3, H, H]
        for k in range(3):
            for t in range(NT):
                base = W_GRU0 + g * GRU_STRIDE + (k * NT + t) * 128
                w_all[t * O : (t + 1) * O, base : base + 128] = wi[k]
        for k in range(3):
            base = W_GRU0 + g * GRU_STRIDE + 3 * NT * 128 + k * 128
            w_all[:, base : base + 128] = wh[k] if k < 2 else 0.5 * wh[k]
    w_all[:, W_U0 : W_U0 + 128] = f(inputs["uf_w"])
    w_all[:, W_U0 + 128 : W_U0 + 256] = f(inputs["ua_w"])
    w_all[:, W_ID : W_ID + 128] = np.eye(128, dtype=np.float32)
    return {"w_all": w_all.astype(ml_dtypes.bfloat16)}


_BUILD_CACHE = {}


def _get_built(depth, arity):
    key = (depth, arity)
    if key not in _BUILD_CACHE:
        _BUILD_CACHE[key] = build(depth, arity)
    return _BUILD_CACHE[key]


def kernel(**inputs) -> np.ndarray:
    depth = int(np.asarray(inputs["depth"]))
    arity = int(np.asarray(inputs["arity"]))
    for bname in ("z2h_b", "h2o_b", "anc_bi", "anc_bh", "frat_bi", "frat_bh", "ua_b", "uf_b"):
        if bname in inputs and np.any(np.asarray(inputs[bname])):
            raise NotImplementedError(f"nonzero bias {bname} not supported")

    nc, nn_ = _get_built(depth, arity)
    w = _prep_weights(inputs)
    z = np.asarray(inputs["z"], dtype=np.float32).reshape(B, I).astype(ml_dtypes.bfloat16)

    in_maps = []
    for c in range(N_CORES):
        im = dict(w)
        im["z"] = np.ascontiguousarray(z[c * B_CORE : (c + 1) * B_CORE])
        in_maps.append(im)

    res = run_bass_kernel_spmd(nc, in_maps, core_ids=list(range(N_CORES)))
    outs = []
    for c in range(N_CORES):
        o = np.asarray(res.results[c]["out"]).astype(np.float32)  # [nn, NS, PO, BT]
        # out[n, s, t*32+o, j] = exp(pred)(batch = s*1024 + t*512 + j, feat o)
        o = o.reshape(nn_, NS, NT, O, BT).transpose(0, 1, 2, 4, 3)
        o = np.ascontiguousarray(o).reshape(nn_, B_CORE, 1, O)
        outs.append(o)
    out = np.concatenate(outs, axis=1)  # [nn, B, 1, O] = exp(pred)
    return np.log(out)


if __name__ == "__main__":
    # smoke test with random inputs
    rng = np.random.default_rng(0)
    ins = {
        "z": rng.standard_normal((B, 1, I)).astype(np.float32),
        "z2h_w": rng.standard_normal((I, H)).astype(np.float32) * 0.08,
        "z2h_b": np.zeros(H, np.float32),
        "h2o_w": rng.standard_normal((H, O)).astype(np.float32) * 0.1,
        "h2o_b": np.zeros(O, np.float32),
        "anc_wi": rng.standard_normal((3, O, H)).astype(np.float32) * 0.1,
        "anc_wh": rng.standard_normal((3, H, H)).astype(np.float32) * 0.08,
        "anc_bi": np.zeros((3, H), np.float32),
        "anc_bh": np.zeros((3, H), np.float32),
        "frat_wi": rng.standard_normal((3, O, H)).astype(np.float32) * 0.1,
        "frat_wh": rng.standard_normal((3, H, H)).astype(np.float32) * 0.08,
        "frat_bi": np.zeros((3, H), np.float32),
        "frat_bh": np.zeros((3, H), np.float32),
        "ua_w": rng.standard_normal((H, H)).astype(np.float32) * 0.08,
        "ua_b": np.zeros(H, np.float32),
        "uf_w": rng.standard_normal((H, H)).astype(np.float32) * 0.08,
        "uf_b": np.zeros(H, np.float32),
        "depth": np.int64(2),
        "arity": np.int64(2),
    }
    out = kernel(**ins)
    print("out shape:", out.shape, "finite:", np.isfinite(out).all())


# revision 4
# speedup vs baseline: 1.0854x; 1.0854x over previous
"""Trainium2 Bass kernel v3 for nn_Decoder (recursive tree GRU decoder).

Self-contained: builds + compiles + runs a Bass/Tile kernel SPMD on 8
NeuronCores, pure data-parallel over the batch dim.

Design:
  - bf16 activations/weights everywhere (DVE 2x/4x modes on SBUF ops, FWL on
    PE, half DMA traffic); PSUM stays fp32.
  - 4 independent streams per core (1024 batch each = 2 tiles of 512), each
    running the tree recursion; streams interleaved at emission so all five
    engines pipeline across streams.
  - GRU elementwise per stream (FD=1024 ops): gates [A|A],[B|B],[C|C],[D|D]
    in 2-bank PSUM tiles; ACT: 2x tanh(gate/2) + 1x tanh(n); DVE:
    m=(trzA+1)*D [1x], na=m+C [1x], z=(trzB+1)/2 [4x ts], pz=z*s [2x],
    h'=pz+n [2x]; s=h-n on GPSIMD (otherwise idle).
  - Output: exp(pred) bf16 DMA'd feature-major (exp is also the softmax
    numerator, so the logits copy is free); host takes log + transposes.
  - z input loaded bf16 via DMA xbar transpose.

Math identical to reference (tanh-only gates: sigmoid(x)=(tanh(x/2)+1)/2,
wh2 pre-scaled by 0.5 on host).
"""

import numpy as np
import ml_dtypes

import concourse.bass as bass
import concourse.mybir as mybir
from concourse import tile
from concourse.bass_utils import run_bass_kernel_spmd

F32 = mybir.dt.float32
BF16 = mybir.dt.bfloat16
AF = mybir.ActivationFunctionType
ALU = mybir.AluOpType

B, I, H, O = 32768, 128, 128, 32
N_CORES = 8
BT = 512            # batch tile (one PSUM bank of fp32)
NT = 2              # tiles per stream (NT*O = 64 pred partitions)
B_CORE = B // N_CORES            # 4096
NS = B_CORE // (BT * NT)         # 4 streams per core
PO = NT * O                      # pred partitions (64)

# weight layout columns
W_Z2H = 0
W_S = 128
W_H2O = W_S + 64
GRU_STRIDE = 3 * NT * 128 + 3 * 128   # wi blocks + wh blocks = 1152
W_GRU0 = W_H2O + NT * 64
W_U0 = W_GRU0 + 2 * GRU_STRIDE
W_ID = W_U0 + 256
W_COLS = W_ID + 128

POOL_S = False      # run the GRU `s = h - n` op on GPSIMD instead of DVE
POOL_DMA = False    # GPSIMD SWDGE DMA: walrus codegen can't handle InstISA here
PE_ACCUM_NA = True  # PE identity-accumulate m into XC (kills `na` DVE op)
TP_BUFS = 3
HP_BUFS = 1

_PE_OPS = ("InstMatmult", "InstLdweights", "InstMatmultMx")


def _split_multi_waits(nc):
    """This container's walrus accepts at most 1 embedded sem wait on most
    instructions (0 on self-loading matmuls) and <=2 on a standalone
    EventSemaphore.  Tile emits multi-waits; split them."""
    for f in nc.m.functions:
        for bb in f.blocks:
            insts = bb.instructions
            new = []
            changed = False
            for ins in insts:
                si = ins.sync_info
                ow = list(si.on_wait) if si is not None and si.on_wait else []
                movable = [w for w in ow if w.wait_reg is None]
                fixed = [w for w in ow if w.wait_reg is not None]
                opc = type(ins).__name__
                limit = 0 if opc in _PE_OPS else 1
                limit = max(0, limit - len(fixed))
                if len(movable) > limit:
                    keep = movable[:limit]
                    move = movable[limit:]
                    for i in range(0, len(move), 2):
                        ev = mybir.InstEventSemaphore(
                            name=f"{ins.name}-wsp{i}",
                            ins=[],
                            outs=[],
                            sync_info=mybir.SyncInfo(
                                on_wait=move[i : i + 2], on_update=[]
                            ),
                        )
                        ev.engine = ins.engine
                        new.append(ev)
                    upd = list(si.on_update) if si.on_update else []
                    ins.sync_info = mybir.SyncInfo(on_wait=fixed + keep, on_update=upd)
                    changed = True
                new.append(ins)
            if changed:
                bb.instructions = new


def _n_nodes(depth, arity):
    n, level = 0, 1
    for _ in range(depth + 1):
        n += level
        level *= arity
    return n


def build(depth, arity, loop_n=1):
    """Build the per-core Bass module.  Returns (nc, n_nodes)."""
    nn_ = _n_nodes(depth, arity)
    nc = bass.Bass(trn_type="TRN2")

    z_d = nc.dram_tensor("z", [B_CORE, I], BF16, kind="ExternalInput")
    w_d = nc.dram_tensor("w_all", [128, W_COLS], BF16, kind="ExternalInput")
    # per node per stream: exp(pred) [part = 2t x 32o, col = batch-in-tile]
    out_d = nc.dram_tensor("out", [nn_, NS, PO, BT], BF16,
                           kind="ExternalOutput")

    with tile.TileContext(nc) as tc:
        with (
            tc.tile_pool(name="wp", bufs=1) as wp,
            tc.tile_pool(name="hp", bufs=HP_BUFS) as hp,
            tc.tile_pool(name="prp", bufs=1) as prp,
            tc.tile_pool(name="tp", bufs=TP_BUFS) as tp,
            tc.tile_pool(name="pp", bufs=3, space="PSUM") as pp,
            tc.tile_pool(name="p1", bufs=2, space="PSUM") as p1,
        ):
            w = wp.tile([128, W_COLS], BF16, tag="w_all")
            nc.sync.dma_start(w[:], w_d[:])

            w_z2h = w[:, W_Z2H : W_Z2H + 128]
            w_S2 = w[0:PO, W_S : W_S + PO]
            def w_h2o(t):
                return w[:, W_H2O + t * PO : W_H2O + (t + 1) * PO]
            def w_gi(g, k, t):
                base = W_GRU0 + g * GRU_STRIDE + (k * NT + t) * 128
                return w[0:PO, base : base + 128]
            def w_gh(g, k):
                base = W_GRU0 + g * GRU_STRIDE + 3 * NT * 128 + k * 128
                return w[:, base : base + 128]
            w_uf = w[:, W_U0 : W_U0 + 128]
            w_ua = w[:, W_U0 + 128 : W_U0 + 256]
            w_id = w[:, W_ID : W_ID + 128]

            from contextlib import ExitStack
            _ls = ExitStack()
            _ls.enter_context(nc.allow_low_precision(reason="bf16 decoder"))
            if loop_n > 1:
                _ls.enter_context(tc.For_i(0, loop_n, 1))

            def stream_gen(sid):
                node_idx = [0]

                # ---- hidden0 = z @ z2h_w  (feature-major) ----
                h_ps = pp.tile([128, 2 * BT], F32, tag="g2")
                for i in range(2):
                    base = sid * (NT * BT) + i * BT
                    zT = tp.tile([128, BT], BF16, tag="zT")
                    nc.sync.dma_start_transpose(zT[:], z_d[base : base + BT, :])
                    nc.tensor.matmul(
                        h_ps[:, i * BT : (i + 1) * BT], w_z2h, zT[:],
                        start=True, stop=True,
                    )
                h0 = hp.tile([128, 2 * BT], BF16, tag=f"h_s{sid}_d{depth}")
                nc.scalar.copy(h0[:], h_ps[:])

                def pred_softmax(h, d, need_probs):
                    n = node_idx[0]
                    node_idx[0] += 1
                    ps = p1.tile([PO, BT], F32, tag="p1")
                    pred_ps = ps[:]
                    for t in range(NT):
                        nc.tensor.matmul(
                            pred_ps, w_h2o(t),
                            h[:, t * BT : (t + 1) * BT],
                            start=(t == 0), stop=(t == NT - 1),
                        )
                    # exp(pred) is both the output (host takes log) and the
                    # softmax numerator.
                    exp_sb = tp.tile([PO, BT], BF16, tag="exp_sb")
                    nc.scalar.activation(exp_sb[:], pred_ps, AF.Exp,
                                         bias=0.0, scale=1.0)
                    probs = None
                    if need_probs:
                        sums_t = p1.tile([PO, BT], F32, tag="p1")
                        sums_ps = sums_t[:]
                        nc.tensor.matmul(sums_ps, w_S2, exp_sb[:],
                                         start=True, stop=True)
                        rbc = tp.tile([PO, BT], BF16, tag="rbc")
                        nc.vector.reciprocal(rbc[:], sums_ps)
                        probs = prp.tile([PO, BT], BF16, tag=f"probs_s{sid}_d{d}")
                        nc.vector.tensor_tensor(out=probs[:], in0=exp_sb[:],
                                                in1=rbc[:], op=ALU.mult)
                    # output DMA from the (otherwise idle) GPSIMD software-DGE
                    # queue so the SP sequencer's in-order HWDGE queue doesn't
                    # head-of-line block across streams.
                    if POOL_DMA:
                        nc.gpsimd.dma_start(out_d[n, sid], exp_sb[:])
                    else:
                        nc.sync.dma_start(out_d[n, sid], exp_sb[:])
                    return probs

                def gru(g, probs, h, d):
                    # gates (fp32 PSUM, [128, 1024] = 2 tiles of 512):
                    #   XA = gi0 + gh0 ; XB = gi1 + gh1 ; XC = gi2 ; XD = 0.5*gh2
                    # r-path: n = tanh(gi2 + r*gh2) = tanh((trzA+1)*XD + XC)
                    # z-path: z = (trzB+1)/2 ; h' = n + z*(h - n)
                    # fill order A, D, C, B: consumers chain trzA -> m(XD) ->
                    # na(XC), and trzB is needed only late (z-path), so this
                    # minimizes PSUM slot residency.
                    XA = pp.tile([128, 2 * BT], F32, tag="g2")
                    for i in range(2):
                        hsl = h[:, i * BT : (i + 1) * BT]
                        c = slice(i * BT, (i + 1) * BT)
                        nc.tensor.matmul(XA[:, c], w_gi(g, 0, i), probs[:], start=True, stop=False)
                        nc.tensor.matmul(XA[:, c], w_gh(g, 0), hsl, start=False, stop=True)
                    trzA = tp.tile([128, 2 * BT], BF16, tag="trzA")
                    nc.scalar.activation(trzA[:], XA[:], AF.Tanh, bias=0.0, scale=0.5)
                    XD = pp.tile([128, 2 * BT], F32, tag="g2")
                    for i in range(2):
                        hsl = h[:, i * BT : (i + 1) * BT]
                        c = slice(i * BT, (i + 1) * BT)
                        nc.tensor.matmul(XD[:, c], w_gh(g, 2), hsl, start=True, stop=True)
                    XC = pp.tile([128, 2 * BT], F32, tag="g2")
                    for i in range(2):
                        c = slice(i * BT, (i + 1) * BT)
                        nc.tensor.matmul(XC[:, c], w_gi(g, 2, i), probs[:],
                                         start=True, stop=not PE_ACCUM_NA)
                    XB = pp.tile([128, 2 * BT], F32, tag="g2")
                    for i in range(2):
                        hsl = h[:, i * BT : (i + 1) * BT]
                        c = slice(i * BT, (i + 1) * BT)
                        nc.tensor.matmul(XB[:, c], w_gi(g, 1, i), probs[:], start=True, stop=False)
                        nc.tensor.matmul(XB[:, c], w_gh(g, 1), hsl, start=False, stop=True)
                    trzB = tp.tile([128, 2 * BT], BF16, tag="trzB")
                    nc.scalar.activation(trzB[:], XB[:], AF.Tanh, bias=0.0, scale=0.5)
                    m = tp.tile([128, 2 * BT], BF16, tag="m")
                    nc.vector.scalar_tensor_tensor(
                        out=m[:], in0=trzA[:], scalar=1.0, in1=XD[:],
                        op0=ALU.add, op1=ALU.mult,
                    )
                    yield
                    if PE_ACCUM_NA:
                        # accumulate m into XC on the PE (identity stationary):
                        # XC <- gi2 + m, so nn = tanh(XC) straight from PSUM.
                        # (costs PE head-of-line blocking; off by default)
                        for i in range(2):
                            c = slice(i * BT, (i + 1) * BT)
                            nc.tensor.matmul(XC[:, c], w_id, m[:, c], start=False, stop=True)
                        na_src = XC[:]
                    else:
                        na = tp.tile([128, 2 * BT], BF16, tag="na")
                        nc.vector.tensor_tensor(out=na[:], in0=m[:], in1=XC[:], op=ALU.add)
                        na_src = na[:]
                    nn_t = tp.tile([128, 2 * BT], BF16, tag="nn_t")
                    nc.scalar.activation(nn_t[:], na_src, AF.Tanh, bias=0.0, scale=1.0)
                    zg = tp.tile([128, 2 * BT], BF16, tag="zg")
                    nc.vector.tensor_scalar(zg[:], trzB[:], 0.5, 0.5,
                                            op0=ALU.mult, op1=ALU.add)
                    s = tp.tile([128, 2 * BT], BF16, tag="s")
                    s_eng = nc.gpsimd if POOL_S else nc.vector
                    s_eng.tensor_tensor(out=s[:], in0=h[:], in1=nn_t[:], op=ALU.subtract)
                    pz = tp.tile([128, 2 * BT], BF16, tag="pz")
                    nc.vector.tensor_tensor(out=pz[:], in0=zg[:], in1=s[:], op=ALU.mult)
                    hn = hp.tile([128, 2 * BT], BF16, tag=f"h_s{sid}_d{d}")
                    nc.vector.tensor_tensor(out=hn[:], in0=pz[:], in1=nn_t[:], op=ALU.add)
                    return hn
                # (gru is a generator: one mid-yield after `m` so other
                # streams' matmuls separate the gate fills from the tail)

                def u_stage(hf, ha, d):
                    U = pp.tile([128, 2 * BT], F32, tag="g2")
                    for i in range(2):
                        c = slice(i * BT, (i + 1) * BT)
                        nc.tensor.matmul(U[:, c], w_uf, hf[:, c], start=True, stop=False)
                        nc.tensor.matmul(U[:, c], w_ua, ha[:, c], start=False, stop=True)
                    ht = hp.tile([128, 2 * BT], BF16, tag=f"h_s{sid}_d{d}")
                    nc.scalar.activation(ht[:], U[:], AF.Tanh, bias=0.0, scale=1.0)
                    return ht

                def rec(h, d, need_probs):
                    probs = pred_softmax(h, d, need_probs or d > 0)
                    yield
                    if d == 0:
                        return probs
                    h1 = yield from gru(0, probs, h, d - 1)
                    yield
                    probs_f = yield from rec(h1, d - 1, arity > 1)
                    hf = h1
                    for sI in range(arity - 1):
                        hf = yield from gru(1, probs_f, hf, d - 1)
                        yield
                        h2 = u_stage(hf, h, d - 1)
                        yield
                        probs_f = yield from rec(h2, d - 1, sI < arity - 2)
                    return probs

                yield from rec(h0, depth, False)

            gens = [stream_gen(s) for s in range(NS)]
            live = list(gens)
            while live:
                for g in list(live):
                    try:
                        next(g)
                    except StopIteration:
                        live.remove(g)

            _ls.close()

    _split_multi_waits(nc)
    return nc, nn_


def _prep_weights(inputs):
    """Host-side weight packing into one [128, W_COLS] bf16 tensor."""
    f = lambda x: np.asarray(x, dtype=np.float32)
    z2h_w = f(inputs["z2h_w"])            # [I, H]
    h2o_w = f(inputs["h2o_w"])            # [H, O]
    w_all = np.zeros((128, W_COLS), np.float32)
    w_all[:, W_Z2H : W_Z2H + 128] = z2h_w
    for t in range(NT):
        w_all[t * O : (t + 1) * O, W_S + t * O : W_S + (t + 1) * O] = 1.0
    for t in range(NT):
        w_all[:, W_H2O + t * PO + t * O : W_H2O + t * PO + (t + 1) * O] = h2o_w
    for g, name in enumerate(("anc", "frat")):
        wi = f(inputs[f"{name}_wi"])      # [3, O, H]
        wh = f(inputs[f"{name}_wh"])      # [3, H, H]
        for k in range(3):
            for t in range(NT):
                base = W_GRU0 + g * GRU_STRIDE + (k * NT + t) * 128
                w_all[t * O : (t + 1) * O, base : base + 128] = wi[k]
        for k in range(3):
            base = W_GRU0 + g * GRU_STRIDE + 3 * NT * 128 + k * 128
            w_all[:, base : base + 128] = wh[k] if k < 2 else 0.5 * wh[k]
    w_all[:, W_U0 : W_U0 + 128] = f(inputs["uf_w"])
    w_all[:, W_U0 + 128 : W_U0 + 256] = f(inputs["ua_w"])
    w_all[:, W_ID : W_ID + 128] = np.eye(128, dtype=np.float32)
    return {"w_all": w_all.astype(ml_dtypes.bfloat16)}


_BUILD_CACHE = {}


def _get_built(depth, arity):
    key = (depth, arity)
    if key not in _BUILD_CACHE:
        _BUILD_CACHE[key] = build(depth, arity)
    return _BUILD_CACHE[key]


def kernel(**inputs) -> np.ndarray:
    depth = int(np.asarray(inputs["depth"]))
    arity = int(np.asarray(inputs["arity"]))
    for bname in ("z2h_b", "h2o_b", "anc_bi", "anc_bh", "frat_bi", "frat_bh", "ua_b", "uf_b"):
        if bname in inputs and np.any(np.asarray(inputs[bname])):
            raise NotImplementedError(f"nonzero bias {bname} not supported")

    nc, nn_ = _get_built(depth, arity)
    w = _prep_weights(inputs)
    z = np.asarray(inputs["z"], dtype=np.float32).reshape(B, I).astype(ml_dtypes.bfloat16)

    in_maps = []
    for c in range(N_CORES):
        im = dict(w)
        im["z"] = np.ascontiguousarray(z[c * B_CORE : (c + 1) * B_CORE])
        in_maps.append(im)

    res = run_bass_kernel_spmd(nc, in_maps, core_ids=list(range(N_CORES)))
    outs = []
    for c in range(N_CORES):
        o = np.asarray(res.results[c]["out"]).astype(np.float32)  # [nn, NS, PO, BT]
        # out[n, s, t*32+o, j] = exp(pred)(batch = s*1024 + t*512 + j, feat o)
        o = o.reshape(nn_, NS, NT, O, BT).transpose(0, 1, 2, 4, 3)
        o = np.ascontiguousarray(o).reshape(nn_, B_CORE, 1, O)
        outs.append(o)
    out = np.concatenate(outs, axis=1)  # [nn, B, 1, O] = exp(pred)
    return np.log(out)


if __name__ == "__main__":
    # smoke test with random inputs
    rng = np.random.default_rng(0)
    ins = {
        "z": rng.standard_normal((B, 1, I)).astype(np.float32),
        "z2h_w": rng.standard_normal((I, H)).astype(np.float32) * 0.08,
        "z2h_b": np.zeros(H, np.float32),
        "h2o_w": rng.standard_normal((H, O)).astype(np.float32) * 0.1,
        "h2o_b": np.zeros(O, np.float32),
        "anc_wi": rng.standard_normal((3, O, H)).astype(np.float32) * 0.1,
        "anc_wh": rng.standard_normal((3, H, H)).astype(np.float32) * 0.08,
        "anc_bi": np.zeros((3, H), np.float32),
        "anc_bh": np.zeros((3, H), np.float32),
        "frat_wi": rng.standard_normal((3, O, H)).astype(np.float32) * 0.1,
        "frat_wh": rng.standard_normal((3, H, H)).astype(np.float32) * 0.08,
        "frat_bi": np.zeros((3, H), np.float32),
        "frat_bh": np.zeros((3, H), np.float32),
        "ua_w": rng.standard_normal((H, H)).astype(np.float32) * 0.08,
        "ua_b": np.zeros(H, np.float32),
        "uf_w": rng.standard_normal((H, H)).astype(np.float32) * 0.08,
        "uf_b": np.zeros(H, np.float32),
        "depth": np.int64(2),
        "arity": np.int64(2),
    }
    out = kernel(**ins)
    print("out shape:", out.shape, "finite:", np.isfinite(out).all())


# revision 6
# speedup vs baseline: 1.1544x; 1.0636x over previous
"""Trainium2 Bass kernel v4 for nn_Decoder (recursive tree GRU decoder).

v4 = v3's 4-stream structure, but elementwise dtypes reverted to fp32
(float32r for matmul operands): HW microbenchmarks show this toolchain's
ant-DVE lowering runs bf16 tensor_tensor/tensor_scalar ~6-10x SLOWER than
fp32 tensor_tensor and stt, the opposite of the cost-model prediction.
Output DMA is fp32 as a consequence (exp(pred) still, host takes log).


Self-contained: builds + compiles + runs a Bass/Tile kernel SPMD on 8
NeuronCores, pure data-parallel over the batch dim.

Design:
  - bf16 activations/weights everywhere (DVE 2x/4x modes on SBUF ops, FWL on
    PE, half DMA traffic); PSUM stays fp32.
  - 4 independent streams per core (1024 batch each = 2 tiles of 512), each
    running the tree recursion; streams interleaved at emission so all five
    engines pipeline across streams.
  - GRU elementwise per stream (FD=1024 ops): gates [A|A],[B|B],[C|C],[D|D]
    in 2-bank PSUM tiles; ACT: 2x tanh(gate/2) + 1x tanh(n); DVE:
    m=(trzA+1)*D [1x], na=m+C [1x], z=(trzB+1)/2 [4x ts], pz=z*s [2x],
    h'=pz+n [2x]; s=h-n on GPSIMD (otherwise idle).
  - Output: exp(pred) bf16 DMA'd feature-major (exp is also the softmax
    numerator, so the logits copy is free); host takes log + transposes.
  - z input loaded bf16 via DMA xbar transpose.

Math identical to reference (tanh-only gates: sigmoid(x)=(tanh(x/2)+1)/2,
wh2 pre-scaled by 0.5 on host).
"""

import numpy as np
import ml_dtypes

import concourse.bass as bass
import concourse.mybir as mybir
from concourse import tile
from concourse.bass_utils import run_bass_kernel_spmd

F32 = mybir.dt.float32
F32R = mybir.dt.float32r
BF16 = mybir.dt.bfloat16
AF = mybir.ActivationFunctionType
ALU = mybir.AluOpType

B, I, H, O = 32768, 128, 128, 32
N_CORES = 8
BT = 512            # batch tile (one PSUM bank of fp32)
NT = 2              # tiles per stream (NT*O = 64 pred partitions)
B_CORE = B // N_CORES            # 4096
NS = B_CORE // (BT * NT)         # 4 streams per core
PO = NT * O                      # pred partitions (64)

# weight layout columns
W_Z2H = 0
W_S = 128
W_H2O = W_S + 64
GRU_STRIDE = 3 * NT * 128 + 3 * 128   # wi blocks + wh blocks = 1152
W_GRU0 = W_H2O + NT * 64
W_U0 = W_GRU0 + 2 * GRU_STRIDE
W_ID = W_U0 + 256
W_COLS = W_ID + 128

POOL_S = False      # run the GRU `s = h - n` op on GPSIMD instead of DVE
POOL_DMA = False    # GPSIMD SWDGE DMA: walrus codegen can't handle InstISA here
PE_ACCUM_NA = True  # PE identity-accumulate m into XC (kills `na` DVE op)
TP_BUFS = 2
HP_BUFS = 1

_PE_OPS = ("InstMatmult", "InstLdweights", "InstMatmultMx")


def _split_multi_waits(nc):
    """This container's walrus accepts at most 1 embedded sem wait on most
    instructions (0 on self-loading matmuls) and <=2 on a standalone
    EventSemaphore.  Tile emits multi-waits; split them."""
    for f in nc.m.functions:
        for bb in f.blocks:
            insts = bb.instructions
            new = []
            changed = False
            for ins in insts:
                si = ins.sync_info
                ow = list(si.on_wait) if si is not None and si.on_wait else []
                movable = [w for w in ow if w.wait_reg is None]
                fixed = [w for w in ow if w.wait_reg is not None]
                opc = type(ins).__name__
                limit = 0 if opc in _PE_OPS else 1
                limit = max(0, limit - len(fixed))
                if len(movable) > limit:
                    keep = movable[:limit]
                    move = movable[limit:]
                    for i in range(0, len(move), 2):
                        ev = mybir.InstEventSemaphore(
                            name=f"{ins.name}-wsp{i}",
                            ins=[],
                            outs=[],
                            sync_info=mybir.SyncInfo(
                                on_wait=move[i : i + 2], on_update=[]
                            ),
                        )
                        ev.engine = ins.engine
                        new.append(ev)
                    upd = list(si.on_update) if si.on_update else []
                    ins.sync_info = mybir.SyncInfo(on_wait=fixed + keep, on_update=upd)
                    changed = True
                new.append(ins)
            if changed:
                bb.instructions = new


def _n_nodes(depth, arity):
    n, level = 0, 1
    for _ in range(depth + 1):
        n += level
        level *= arity
    return n


def build(depth, arity, loop_n=1):
    """Build the per-core Bass module.  Returns (nc, n_nodes)."""
    nn_ = _n_nodes(depth, arity)
    nc = bass.Bass(trn_type="TRN2")

    z_d = nc.dram_tensor("z", [B_CORE, I], BF16, kind="ExternalInput")
    w_d = nc.dram_tensor("w_all", [128, W_COLS], F32R, kind="ExternalInput")
    wz_d = nc.dram_tensor("w_z2h_bf16", [128, 256], BF16, kind="ExternalInput")
    # per node per stream: exp(pred) [part = 2t x 32o, col = batch-in-tile]
    out_d = nc.dram_tensor("out", [nn_, NS, PO, BT], F32,
                           kind="ExternalOutput")

    with tile.TileContext(nc) as tc:
        with (
            tc.tile_pool(name="wp", bufs=1) as wp,
            tc.tile_pool(name="hp", bufs=HP_BUFS) as hp,
            tc.tile_pool(name="prp", bufs=1) as prp,
            tc.tile_pool(name="tp", bufs=TP_BUFS) as tp,
            tc.tile_pool(name="pp", bufs=3, space="PSUM") as pp,
            tc.tile_pool(name="p1", bufs=2, space="PSUM") as p1,
        ):
            w = wp.tile([128, W_COLS], F32R, tag="w_all")
            nc.sync.dma_start(w[:], w_d[:])
            wz = wp.tile([128, 256], BF16, tag="wz")
            nc.sync.dma_start(wz[:], wz_d[:])

            w_z2h = w[:, W_Z2H : W_Z2H + 128]
            w_S2 = w[0:PO, W_S : W_S + PO]
            def w_h2o(t):
                return w[:, W_H2O + t * PO : W_H2O + (t + 1) * PO]
            def w_gi(g, k, t):
                base = W_GRU0 + g * GRU_STRIDE + (k * NT + t) * 128
                return w[0:PO, base : base + 128]
            def w_gh(g, k):
                base = W_GRU0 + g * GRU_STRIDE + 3 * NT * 128 + k * 128
                return w[:, base : base + 128]
            w_uf = w[:, W_U0 : W_U0 + 128]
            w_ua = w[:, W_U0 + 128 : W_U0 + 256]
            w_id = w[:, W_ID : W_ID + 128]

            from contextlib import ExitStack
            _ls = ExitStack()
            _ls.enter_context(nc.allow_low_precision(reason="bf16 decoder"))
            if loop_n > 1:
                _ls.enter_context(tc.For_i(0, loop_n, 1))

            def stream_gen(sid):
                node_idx = [0]

                # ---- hidden0 = z @ z2h_w  (feature-major) ----
                h_ps = pp.tile([128, 2 * BT], F32, tag="g2")
                for i in range(2):
                    base = sid * (NT * BT) + i * BT
                    zT = tp.tile([128, BT], BF16, tag="zT", bufs=1)
                    nc.sync.dma_start_transpose(zT[:], z_d[base : base + BT, :])
                    nc.tensor.matmul(
                        h_ps[:, i * BT : (i + 1) * BT], wz[:, 0:128], zT[:],
                        start=True, stop=True,
                    )
                h0 = hp.tile([128, 2 * BT], F32R, tag=f"h_s{sid}_d{depth}")
                nc.scalar.copy(h0[:], h_ps[:])

                def pred_softmax(h, d, need_probs):
                    n = node_idx[0]
                    node_idx[0] += 1
                    ps = p1.tile([PO, BT], F32, tag="p1")
                    pred_ps = ps[:]
                    for t in range(NT):
                        nc.tensor.matmul(
                            pred_ps, w_h2o(t),
                            h[:, t * BT : (t + 1) * BT],
                            start=(t == 0), stop=(t == NT - 1),
                        )
                    # exp(pred) is both the output (host takes log) and the
                    # softmax numerator.
                    exp_sb = tp.tile([PO, BT], F32R, tag="exp_sb")
                    nc.scalar.activation(exp_sb[:], pred_ps, AF.Exp,
                                         bias=0.0, scale=1.0)
                    probs = None
                    if need_probs:
                        sums_t = p1.tile([PO, BT], F32, tag="p1")
                        sums_ps = sums_t[:]
                        nc.tensor.matmul(sums_ps, w_S2, exp_sb[:],
                                         start=True, stop=True)
                        rbc = tp.tile([PO, BT], F32, tag="rbc", bufs=1)
                        nc.vector.reciprocal(rbc[:], sums_ps)
                        probs = prp.tile([PO, BT], F32R, tag=f"probs_s{sid}_d{d}")
                        nc.vector.tensor_tensor(out=probs[:], in0=exp_sb[:],
                                                in1=rbc[:], op=ALU.mult)
                    # output DMA from the (otherwise idle) GPSIMD software-DGE
                    # queue so the SP sequencer's in-order HWDGE queue doesn't
                    # head-of-line block across streams.
                    if POOL_DMA:
                        nc.gpsimd.dma_start(out_d[n, sid], exp_sb[:])
                    else:
                        nc.sync.dma_start(out_d[n, sid], exp_sb[:].bitcast(F32))
                    return probs

                def gru(g, probs, h, d):
                    # gates (fp32 PSUM, [128, 1024] = 2 tiles of 512):
                    #   XA = gi0 + gh0 ; XB = gi1 + gh1 ; XC = gi2 ; XD = 0.5*gh2
                    # r-path: n = tanh(gi2 + r*gh2) = tanh((trzA+1)*XD + XC)
                    # z-path: z = (trzB+1)/2 ; h' = n + z*(h - n)
                    # fill order A, D, C, B: consumers chain trzA -> m(XD) ->
                    # na(XC), and trzB is needed only late (z-path), so this
                    # minimizes PSUM slot residency.
                    XA = pp.tile([128, 2 * BT], F32, tag="g2")
                    for i in range(2):
                        hsl = h[:, i * BT : (i + 1) * BT]
                        c = slice(i * BT, (i + 1) * BT)
                        nc.tensor.matmul(XA[:, c], w_gi(g, 0, i), probs[:], start=True, stop=False)
                        nc.tensor.matmul(XA[:, c], w_gh(g, 0), hsl, start=False, stop=True)
                    trzA = tp.tile([128, 2 * BT], F32, tag="trzA")
                    nc.scalar.activation(trzA[:], XA[:], AF.Tanh, bias=0.0, scale=0.5)
                    XD = pp.tile([128, 2 * BT], F32, tag="g2")
                    for i in range(2):
                        hsl = h[:, i * BT : (i + 1) * BT]
                        c = slice(i * BT, (i + 1) * BT)
                        nc.tensor.matmul(XD[:, c], w_gh(g, 2), hsl, start=True, stop=True)
                    XC = pp.tile([128, 2 * BT], F32, tag="g2")
                    for i in range(2):
                        c = slice(i * BT, (i + 1) * BT)
                        nc.tensor.matmul(XC[:, c], w_gi(g, 2, i), probs[:],
                                         start=True, stop=not PE_ACCUM_NA)
                    XB = pp.tile([128, 2 * BT], F32, tag="g2")
                    for i in range(2):
                        hsl = h[:, i * BT : (i + 1) * BT]
                        c = slice(i * BT, (i + 1) * BT)
                        nc.tensor.matmul(XB[:, c], w_gi(g, 1, i), probs[:], start=True, stop=False)
                        nc.tensor.matmul(XB[:, c], w_gh(g, 1), hsl, start=False, stop=True)
                    trzB = tp.tile([128, 2 * BT], F32, tag="trzB")
                    nc.scalar.activation(trzB[:], XB[:], AF.Tanh, bias=0.0, scale=0.5)
                    m = tp.tile([128, 2 * BT], F32R, tag="m", bufs=1)
                    nc.vector.scalar_tensor_tensor(
                        out=m[:], in0=trzA[:], scalar=1.0, in1=XD[:],
                        op0=ALU.add, op1=ALU.mult,
                    )
                    yield
                    if PE_ACCUM_NA:
                        # accumulate m into XC on the PE (identity stationary):
                        # XC <- gi2 + m, so nn = tanh(XC) straight from PSUM.
                        # (costs PE head-of-line blocking; off by default)
                        for i in range(2):
                            c = slice(i * BT, (i + 1) * BT)
                            nc.tensor.matmul(XC[:, c], w_id, m[:, c], start=False, stop=True)
                        na_src = XC[:]
                    else:
                        na = tp.tile([128, 2 * BT], F32, tag="na")
                        nc.vector.tensor_tensor(out=na[:], in0=m[:], in1=XC[:], op=ALU.add)
                        na_src = na[:]
                    nn_t = tp.tile([128, 2 * BT], F32, tag="nn_t")
                    nc.scalar.activation(nn_t[:], na_src, AF.Tanh, bias=0.0, scale=1.0)
                    s = tp.tile([128, 2 * BT], F32, tag="s")
                    s_eng = nc.gpsimd if POOL_S else nc.vector
                    s_eng.tensor_tensor(out=s[:], in0=h[:].bitcast(F32), in1=nn_t[:], op=ALU.subtract)
                    u1 = tp.tile([128, 2 * BT], F32, tag="u1")
                    nc.vector.scalar_tensor_tensor(
                        out=u1[:], in0=trzB[:], scalar=1.0, in1=s[:],
                        op0=ALU.add, op1=ALU.mult,
                    )
                    hn = hp.tile([128, 2 * BT], F32R, tag=f"h_s{sid}_d{d}")
                    nc.vector.scalar_tensor_tensor(
                        out=hn[:], in0=u1[:], scalar=0.5, in1=nn_t[:],
                        op0=ALU.mult, op1=ALU.add,
                    )
                    return hn
                # (gru is a generator: one mid-yield after `m` so other
                # streams' matmuls separate the gate fills from the tail)

                def u_stage(hf, ha, d):
                    U = pp.tile([128, 2 * BT], F32, tag="g2")
                    for i in range(2):
                        c = slice(i * BT, (i + 1) * BT)
                        nc.tensor.matmul(U[:, c], w_uf, hf[:, c], start=True, stop=False)
                        nc.tensor.matmul(U[:, c], w_ua, ha[:, c], start=False, stop=True)
                    ht = hp.tile([128, 2 * BT], F32R, tag=f"h_s{sid}_d{d}")
                    nc.scalar.activation(ht[:], U[:], AF.Tanh, bias=0.0, scale=1.0)
                    return ht

                def rec(h, d, need_probs):
                    probs = pred_softmax(h, d, need_probs or d > 0)
                    yield
                    if d == 0:
                        return probs
                    h1 = yield from gru(0, probs, h, d - 1)
                    yield
                    probs_f = yield from rec(h1, d - 1, arity > 1)
                    hf = h1
                    for sI in range(arity - 1):
                        hf = yield from gru(1, probs_f, hf, d - 1)
                        yield
                        h2 = u_stage(hf, h, d - 1)
                        yield
                        probs_f = yield from rec(h2, d - 1, sI < arity - 2)
                    return probs

                yield from rec(h0, depth, False)

            gens = [stream_gen(s) for s in range(NS)]
            live = list(gens)
            while live:
                for g in list(live):
                    try:
                        next(g)
                    except StopIteration:
                        live.remove(g)

            _ls.close()

    _split_multi_waits(nc)
    return nc, nn_


def _prep_weights(inputs):
    """Host-side weight packing into one [128, W_COLS] bf16 tensor."""
    f = lambda x: np.asarray(x, dtype=np.float32)
    z2h_w = f(inputs["z2h_w"])            # [I, H]
    h2o_w = f(inputs["h2o_w"])            # [H, O]
    w_all = np.zeros((128, W_COLS), np.float32)
    w_all[:, W_Z2H : W_Z2H + 128] = z2h_w
    for t in range(NT):
        w_all[t * O : (t + 1) * O, W_S + t * O : W_S + (t + 1) * O] = 1.0
    for t in range(NT):
        w_all[:, W_H2O + t * PO + t * O : W_H2O + t * PO + (t + 1) * O] = h2o_w
    for g, name in enumerate(("anc", "frat")):
        wi = f(inputs[f"{name}_wi"])      # [3, O, H]
        wh = f(inputs[f"{name}_wh"])      # [3, H, H]
        for k in range(3):
            for t in range(NT):
                base = W_GRU0 + g * GRU_STRIDE + (k * NT + t) * 128
                w_all[t * O : (t + 1) * O, base : base + 128] = wi[k]
        for k in range(3):
            base = W_GRU0 + g * GRU_STRIDE + 3 * NT * 128 + k * 128
            w_all[:, base : base + 128] = wh[k] if k < 2 else 0.5 * wh[k]
    w_all[:, W_U0 : W_U0 + 128] = f(inputs["uf_w"])
    w_all[:, W_U0 + 128 : W_U0 + 256] = f(inputs["ua_w"])
    w_all[:, W_ID : W_ID + 128] = np.eye(128, dtype=np.float32)
    return {"w_all": w_all.astype(np.float32),
            "w_z2h_bf16": np.concatenate(
                [w_all[:, W_Z2H:W_Z2H+128], np.eye(128, dtype=np.float32)],
                axis=1).astype(ml_dtypes.bfloat16)}


_BUILD_CACHE = {}


def _get_built(depth, arity):
    key = (depth, arity)
    if key not in _BUILD_CACHE:
        _BUILD_CACHE[key] = build(depth, arity)
    return _BUILD_CACHE[key]


def kernel(**inputs) -> np.ndarray:
    depth = int(np.asarray(inputs["depth"]))
    arity = int(np.asarray(inputs["arity"]))
    for bname in ("z2h_b", "h2o_b", "anc_bi", "anc_bh", "frat_bi", "frat_bh", "ua_b", "uf_b"):
        if bname in inputs and np.any(np.asarray(inputs[bname])):
            raise NotImplementedError(f"nonzero bias {bname} not supported")

    nc, nn_ = _get_built(depth, arity)
    w = _prep_weights(inputs)
    z = np.asarray(inputs["z"], dtype=np.float32).reshape(B, I).astype(ml_dtypes.bfloat16)

    in_maps = []
    for c in range(N_CORES):
        im = dict(w)
        im["z"] = np.ascontiguousarray(z[c * B_CORE : (c + 1) * B_CORE])
        in_maps.append(im)

    res = run_bass_kernel_spmd(nc, in_maps, core_ids=list(range(N_CORES)))
    outs = []
    for c in range(N_CORES):
        o = np.asarray(res.results[c]["out"]).astype(np.float32)  # [nn, NS, PO, BT]
        # out[n, s, t*32+o, j] = exp(pred)(batch = s*1024 + t*512 + j, feat o)
        o = o.reshape(nn_, NS, NT, O, BT).transpose(0, 1, 2, 4, 3)
        o = np.ascontiguousarray(o).reshape(nn_, B_CORE, 1, O)
        outs.append(o)
    out = np.concatenate(outs, axis=1)  # [nn, B, 1, O] = exp(pred)
    return np.log(out)


if __name__ == "__main__":
    # smoke test with random inputs
    rng = np.random.default_rng(0)
    ins = {
        "z": rng.standard_normal((B, 1, I)).astype(np.float32),
        "z2h_w": rng.standard_normal((I, H)).astype(np.float32) * 0.08,
        "z2h_b": np.zeros(H, np.float32),
        "h2o_w": rng.standard_normal((H, O)).astype(np.float32) * 0.1,
        "h2o_b": np.zeros(O, np.float32),
        "anc_wi": rng.standard_normal((3, O, H)).astype(np.float32) * 0.1,
        "anc_wh": rng.standard_normal((3, H, H)).astype(np.float32) * 0.08,
        "anc_bi": np.zeros((3, H), np.float32),
        "anc_bh": np.zeros((3, H), np.float32),
        "frat_wi": rng.standard_normal((3, O, H)).astype(np.float32) * 0.1,
        "frat_wh": rng.standard_normal((3, H, H)).astype(np.float32) * 0.08,
        "frat_bi": np.zeros((3, H), np.float32),
        "frat_bh": np.zeros((3, H), np.float32),
        "ua_w": rng.standard_normal((H, H)).astype(np.float32) * 0.08,
        "ua_b": np.zeros(H, np.float32),
        "uf_w": rng.standard_normal((H, H)).astype(np.float32) * 0.08,
        "uf_b": np.zeros(H, np.float32),
        "depth": np.int64(2),
        "arity": np.int64(2),
    }
    out = kernel(**ins)
    print("out shape:", out.shape, "finite:", np.isfinite(out).all())
